# revision 1
# baseline (speedup 1.0000x reference)
"""Trainium2 Bass kernel for BarlowTwinsLoss (nn_BarlowTwinsLoss_11038065951192).

Full inputs: e_q, tau [16384, 2048] f32. Output: scalar f32 loss.

Strategy (data-parallel over the batch axis, 8 NeuronCores):
  - each core holds a [2048, 2048] row-shard of e_q and tau
  - one pass over the shard computes 5 per-feature partial sums in PSUM via
    ones-vector matmuls: S1e, S1t, S2e, S2t, Set (length-2048 each; matmul
    outputs may only target PSUM partitions {0,32,64}, so stats pack as
    partition group g = s//2, bank = (s%2)*4 + chunk)
  - the 5x2048 f32 partial stats are ReduceScattered across the 8 cores
    (40 KB in, 5 KB out per core: core r receives the global stats for
    features [256r, 256r+256))
  - a small single-partition epilogue computes mean/std/diag-corr and each
    core's partial loss over its 256 features; the host sums the 8 partials
    (the "unshard" step for a feature-sharded loss).

The module is self-contained: it builds + compiles the Bass graph on first
call and caches the jitted PJRT executable for repeat calls.

Hardware pitfalls baked into this design (found by probing; the simulator
accepts all of them but silicon does not):
  - DVE tensor_tensor with f32 inputs and bf16 output produces garbage ->
    multiply the bf16 copies instead
  - InstTensorTensorReduce crashes the exec unit -> tensor_mul + reduce_sum
  - ACT reading bf16 input crashes the exec unit -> keep ACT on f32 inputs
  - DMA cannot read PSUM -> stage through SBUF with a compute-engine copy
"""

import numpy as np

N_FULL = 16384
D = 2048
N_CORES = 8
N_SHARD = N_FULL // N_CORES  # 2048 rows per core
P = 128
N_TILES = N_SHARD // P  # 16
CHUNK = 512
N_CHUNKS = D // CHUNK  # 4
NSTATS = 5  # S1e, S1t, S2e, S2t, Set
FSHARD = D // N_CORES  # 256 features per core after ReduceScatter
EPS = 1e-9

_CACHE = {}


def _build_nc(repeat=1, collective=True, loop=None):
    import contextlib

    import concourse.bacc as bacc
    import concourse.tile as tile
    from concourse import mybir

    f32 = mybir.dt.float32
    bf16 = mybir.dt.bfloat16
    Act = mybir.ActivationFunctionType
    Alu = mybir.AluOpType

    nc = bacc.Bacc(
        "TRN2",
        target_bir_lowering=False,
        debug=False,
        enable_asserts=False,
        num_devices=N_CORES if collective else 1,
    )
    eq_d = nc.dram_tensor("e_q", [N_SHARD, D], f32, kind="ExternalInput")
    ta_d = nc.dram_tensor("tau", [N_SHARD, D], f32, kind="ExternalInput")
    out_d = nc.dram_tensor("out", [1, 1], f32, kind="ExternalOutput")

    with tile.TileContext(nc) as tc:
        with (
            tc.tile_pool(name="io", bufs=3) as io,
            tc.tile_pool(name="bfp", bufs=2) as bfp,
            tc.tile_pool(name="misc", bufs=1) as misc,
            tc.tile_pool(name="ep", bufs=1) as ep,
            tc.tile_pool(name="psp", bufs=1, space="PSUM") as psp,
            tc.tile_pool(name="dram", bufs=1, space="DRAM") as dram,
        ):
            ones_bf = misc.tile([P, 1], bf16)
            nc.gpsimd.memset(ones_bf[:], 1.0)
            zero_b = misc.tile([P, 1], f32)
            nc.gpsimd.memset(zero_b[:], 0.0)

            # stats accumulate in PSUM; matmuls only write rows {0,32,64} --
            # zero the tile once so the whole-tile PSUM->SBUF staging copy
            # reads initialized memory (start=True re-inits written regions
            # on every pass).
            psum_stats = psp.tile([65, 2 * N_CHUNKS * CHUNK], f32, tag="stats")
            nc.vector.memset(psum_stats[:], 0.0)

            for _rep in range(repeat):
                loop_cm = (
                    tc.For_i(
                        0,
                        loop,
                        1,
                        hint_engines=(
                            mybir.EngineType.PE,
                            mybir.EngineType.DVE,
                            mybir.EngineType.Activation,
                            mybir.EngineType.SP,
                        ),
                    )
                    if loop is not None
                    else contextlib.nullcontext()
                )
                # feature-sharded stats for the collective: row r holds this
                # core's partial stats for features [256r, 256r+256)
                cc_in = dram.tile(
                    [N_CORES, NSTATS, FSHARD], f32, tag=f"cc_in{_rep}", name="cc_in"
                )
                rs_out = dram.tile(
                    [1, NSTATS, FSHARD], f32, tag=f"rs_out{_rep}", name="rs_out"
                )
                with contextlib.ExitStack() as _stack:
                    _stack.enter_context(loop_cm)

                    for i in range(N_TILES):
                        e_t = io.tile([P, D], f32, tag="e")
                        t_t = io.tile([P, D], f32, tag="t")
                        nc.sync.dma_start(e_t[:], eq_d[i * P : (i + 1) * P, :])
                        nc.sync.dma_start(t_t[:], ta_d[i * P : (i + 1) * P, :])

                        e_bf = bfp.tile([P, D], bf16, tag="e_bf")
                        t_bf = bfp.tile([P, D], bf16, tag="t_bf")
                        e2_bf = bfp.tile([P, D], bf16, tag="e2_bf")
                        t2_bf = bfp.tile([P, D], bf16, tag="t2_bf")
                        et_bf = bfp.tile([P, D], bf16, tag="et_bf")

                        nc.vector.tensor_copy(e_bf[:], e_t[:])
                        nc.vector.tensor_copy(t_bf[:], t_t[:])
                        nc.scalar.activation(
                            e2_bf[:], e_t[:], Act.Square, bias=zero_b[:]
                        )
                        nc.scalar.activation(
                            t2_bf[:], t_t[:], Act.Square, bias=zero_b[:]
                        )
                        nc.vector.tensor_mul(et_bf[:], e_bf[:], t_bf[:])

                        for s, src in enumerate((e_bf, t_bf, e2_bf, t2_bf, et_bf)):
                            g, sl = divmod(s, 2)
                            for c in range(N_CHUNKS):
                                col = (sl * N_CHUNKS + c) * CHUNK
                                nc.tensor.matmul(
                                    psum_stats[
                                        32 * g : 32 * g + 1, col : col + CHUNK
                                    ],
                                    ones_bf[:, 0:1],
                                    src[:, c * CHUNK : (c + 1) * CHUNK],
                                    start=(i == 0),
                                    stop=(i == N_TILES - 1),
                                )

                    # PSUM -> SBUF staging (DMA cannot read PSUM). Split the
                    # free range across DVE and ACT so the copies overlap.
                    sb_stats = ep.tile(
                        [65, 2 * N_CHUNKS * CHUNK], f32, tag="sb_stats"
                    )
                    nc.vector.tensor_copy(
                        sb_stats[:, : N_CHUNKS * CHUNK],
                        psum_stats[:, : N_CHUNKS * CHUNK],
                    )
                    nc.scalar.copy(
                        sb_stats[:, N_CHUNKS * CHUNK :],
                        psum_stats[:, N_CHUNKS * CHUNK :],
                    )

                    # scatter the staged stats into cc_in, one DMA per PSUM
                    # partition group. Per-rank stat slot s' = sl*3 + g (so a
                    # group's pair of stats is a stride-3 slice of cc_in's
                    # stat axis, letting src/dst iteration orders agree).
                    for g in range(3):
                        n_s = 2 if g < 2 else 1
                        dst = cc_in[:, g::3, :].rearrange("r s m -> s r m")
                        src = sb_stats[
                            32 * g : 32 * g + 1, : n_s * D
                        ].rearrange("p (s r m) -> p s r m", s=n_s, r=N_CORES)
                        nc.sync.dma_start(dst, src)

                    if collective:
                        nc.gpsimd.collective_compute(
                            "ReduceScatter",
                            Alu.add,
                            replica_groups=[list(range(N_CORES))],
                            ins=[cc_in.opt()],
                            outs=[rs_out.opt()],
                        )
                    else:  # timing variant: placeholder copy instead of RS
                        nc.sync.dma_start(rs_out[:], cc_in[0:1])

                    # global stats for this core's 256 features, all on one
                    # partition: [1, 5*256]
                    st = ep.tile([1, NSTATS * FSHARD], f32, tag="st")
                    nc.sync.dma_start(st[:], rs_out[:])

                    # per-rank stat slots follow s' = sl*3 + g (see scatter)
                    A = st[:, 0 * FSHARD : 1 * FSHARD]  # S1e (g0, sl0)
                    C = st[:, 1 * FSHARD : 2 * FSHARD]  # S2e (g1, sl0)
                    E = st[:, 2 * FSHARD : 3 * FSHARD]  # Set (g2, sl0)
                    B = st[:, 3 * FSHARD : 4 * FSHARD]  # S1t (g0, sl1)
                    Dq = st[:, 4 * FSHARD : 5 * FSHARD]  # S2t (g1, sl1)

                    sh = [1, FSHARD]
                    zb = zero_b[0:1, 0:1]
                    aa = ep.tile(sh, f32, tag="aa")
                    bb = ep.tile(sh, f32, tag="bb")
                    ve = ep.tile(sh, f32, tag="ve")
                    vt = ep.tile(sh, f32, tag="vt")
                    stde = ep.tile(sh, f32, tag="stde")
                    stdt = ep.tile(sh, f32, tag="stdt")
                    amt = ep.tile(sh, f32, tag="amt")
                    cov = ep.tile(sh, f32, tag="cov")
                    den = ep.tile(sh, f32, tag="den")
                    rec = ep.tile(sh, f32, tag="rec")
                    cr = ep.tile(sh, f32, tag="cr")
                    ccl = ep.tile(sh, f32, tag="ccl")
                    rr = ep.tile(sh, f32, tag="rr")
                    r2 = ep.tile(sh, f32, tag="r2")
                    ls = ep.tile([1, 1], f32, tag="ls")

                    inv_n = 1.0 / N_FULL
                    # sum((x-mean)^2) = S2 - S1^2/N ; std = max(sqrt(./(N-1)), eps)
                    nc.vector.tensor_mul(aa[:], A, A)
                    nc.vector.scalar_tensor_tensor(
                        ve[:], aa[:], -inv_n, C, Alu.mult, Alu.add
                    )
                    nc.scalar.activation(
                        stde[:], ve[:], Act.Sqrt, bias=zb, scale=1.0 / (N_FULL - 1)
                    )
                    nc.vector.tensor_scalar_max(stde[:], stde[:], EPS)
                    nc.vector.tensor_mul(bb[:], B, B)
                    nc.vector.scalar_tensor_tensor(
                        vt[:], bb[:], -inv_n, Dq, Alu.mult, Alu.add
                    )
                    nc.scalar.activation(
                        stdt[:], vt[:], Act.Sqrt, bias=zb, scale=1.0 / (N_FULL - 1)
                    )
                    nc.vector.tensor_scalar_max(stdt[:], stdt[:], EPS)
                    # cov = Set - S1e*S1t/N ; c = cov / (stde*stdt) / (N+eps)
                    nc.vector.scalar_tensor_tensor(
                        amt[:], A, inv_n, B, Alu.mult, Alu.mult
                    )
                    nc.vector.tensor_sub(cov[:], E, amt[:])
                    nc.vector.tensor_mul(den[:], stde[:], stdt[:])
                    nc.vector.reciprocal(rec[:], den[:])
                    nc.vector.scalar_tensor_tensor(
                        cr[:], cov[:], 1.0 / (N_FULL + EPS), rec[:], Alu.mult, Alu.mult
                    )
                    # clip, r = 1 - c, partial loss = sum(r^2)
                    nc.vector.tensor_scalar(
                        ccl[:], cr[:], -1.0 + EPS, 1.0 - EPS, Alu.max, Alu.min
                    )
                    nc.vector.tensor_scalar(
                        rr[:], ccl[:], -1.0, 1.0, Alu.mult, Alu.add
                    )
                    nc.vector.tensor_mul(r2[:], rr[:], rr[:])
                    nc.vector.reduce_sum(ls[:], r2[:], axis=mybir.AxisListType.X)
                    nc.sync.dma_start(out_d[:], ls[:])

    nc.compile()
    return nc


class _Exec:
    """Cached PJRT executable for the SPMD kernel (mirrors
    concourse.bass2jax.run_bass_via_pjrt's multi-core branch, but keeps the
    jitted callable so repeat invocations don't recompile)."""

    def __init__(self, nc):
        import jax
        from jax.experimental.shard_map import shard_map
        from jax.sharding import Mesh, PartitionSpec

        from concourse import bass2jax, mybir

        bass2jax.install_neuronx_cc_hook()
        self.nc = nc
        partition_name = (
            nc.partition_id_tensor.name if nc.partition_id_tensor else None
        )

        in_names, out_names, out_avals, zero_outs = [], [], [], []
        for alloc in nc.m.functions[0].allocations:
            if not isinstance(alloc, mybir.MemoryLocationSet):
                continue
            assert alloc.memorylocations
            name = alloc.memorylocations[0].name
            if alloc.kind == "ExternalInput":
                if name != partition_name:
                    in_names.append(name)
            elif alloc.kind == "ExternalOutput":
                shape = tuple(alloc.tensor_shape)
                dtype = mybir.dt.np(alloc.dtype)
                out_names.append(name)
                out_avals.append(jax.core.ShapedArray(shape, dtype))
                zero_outs.append(np.zeros(shape, dtype))

        self.in_names = list(in_names)
        self.out_names = list(out_names)
        self.out_avals = out_avals
        self.zero_outs = zero_outs
        n_params = len(in_names)
        n_outs = len(out_names)

        in_names_full = list(in_names) + list(out_names)
        if partition_name is not None:
            in_names_full.append(partition_name)

        def _body(*args):
            operands = list(args)
            if partition_name is not None:
                operands.append(bass2jax.partition_id_tensor())
            outs = bass2jax._bass_exec_p.bind(
                *operands,
                out_avals=tuple(out_avals),
                in_names=tuple(in_names_full),
                out_names=tuple(out_names),
                lowering_input_output_aliases=(),
                sim_require_finite=True,
                sim_require_nnan=True,
                nc=nc,
            )
            return tuple(outs)

        devices = jax.devices()[:N_CORES]
        assert len(devices) == N_CORES, f"need {N_CORES} devices, got {len(devices)}"
        self.mesh = Mesh(np.asarray(devices), ("core",))
        in_specs = (PartitionSpec("core"),) * (n_params + n_outs)
        out_specs = (PartitionSpec("core"),) * n_outs
        donate = tuple(range(n_params, n_params + n_outs))
        self.sharded = jax.jit(
            shard_map(
                _body,
                mesh=self.mesh,
                in_specs=in_specs,
                out_specs=out_specs,
                check_rep=False,
            ),
            donate_argnums=donate,
            keep_unused=True,
        )

    def concat_zeros(self):
        return [
            np.zeros((N_CORES * z.shape[0], *z.shape[1:]), z.dtype)
            for z in self.zero_outs
        ]

    def run(self, in_map):
        """in_map: name -> full (already concat-along-axis0) array."""
        ins = [in_map[name] for name in self.in_names]
        outs = self.sharded(*ins, *self.concat_zeros())
        return {
            name: np.asarray(outs[i]).reshape(
                N_CORES, *self.out_avals[i].shape
            )
            for i, name in enumerate(self.out_names)
        }


def _get_exec(repeat=1):
    key = ("exec", repeat)
    if key not in _CACHE:
        _CACHE[key] = _Exec(_build_nc(repeat))
    return _CACHE[key]


def kernel(e_q, tau):
    e_q = np.ascontiguousarray(np.asarray(e_q), dtype=np.float32)
    tau = np.ascontiguousarray(np.asarray(tau), dtype=np.float32)
    assert e_q.shape == (N_FULL, D) and tau.shape == (N_FULL, D)
    ex = _get_exec()
    # row-sharding across cores: the concatenation of the 8 shards along
    # axis 0 is just the full array, so pass it through unchanged.
    outs = ex.run({"e_q": e_q, "tau": tau})
    # each core holds the partial loss over its 256 features; the sum over
    # cores is the unshard/gather step for the feature-sharded loss.
    loss = outs["out"][:, 0, 0].astype(np.float64).sum()
    return np.asarray(loss, dtype=np.float32)



# revision 7
# speedup vs baseline: 13.1702x; 13.1702x over previous
"""Trainium2 Bass kernel for BarlowTwinsLoss (nn_BarlowTwinsLoss_11038065951192).

Full inputs: e_q, tau [16384, 2048] f32. Output: scalar f32 loss.

Strategy — feature-sharded, collective-free (8 NeuronCores):
  The loss needs only 5 per-feature reductions over the batch (S1e, S1t,
  S2e, S2t, Set) plus a cheap per-feature epilogue. Instead of batch
  sharding + ReduceScatter of the D-length stats (the baseline; the 8-way
  ReduceScatter alone measured ~450 us of the 546 us pass), core c owns
  features [256c, 256c+256) and reads the FULL batch for its slice: the
  same 32 MiB/core of HBM traffic, but stats are complete locally and no
  collective is needed. Each core emits its partial loss over its 256
  features; the host sums the 8 scalars (the unshard step).

Host-side shard layout: A_c[128 i + p, 256 b + m] = x[1024 i + 128 b + p,
256 c + m], i.e. each core's [2048, 2048] f32 shard is permuted so every
[128, 2048] row-tile is DMA'd with contiguous 8 KiB partition lines — the
pattern measured at the ~366 GB/s/core HBM roofline (a strided column-slice
load measured 18% slower; this kernel is memory-bound, so the DMA stream IS
the runtime).

Per tile [128 rows x (8 blocks x 256 features)]:
  - DVE copies e, t to bf16; ACT squares e, t to bf16; DVE multiplies e*t
    (bf16) — all under the DMA stream
  - 40 ones-matmuls (5 stats x 8 row-blocks) of 256 output columns
    accumulate directly into one [1, 256] PSUM slot per stat (matmul
    outputs may only target PSUM partitions {0,32,64}; stat s -> row 32g,
    bank-start col 512 sl with g, sl = divmod(s, 2)); no block fold needed
Tail (once): one [65, 1024] PSUM->SBUF staging copy, one SBUF->SBUF gather
DMA that brings the 5 stats to partition 0, then the 256-feature epilogue
(mean/var/corr/clip/loss) and a [1,1] DMA out.

Hardware pitfalls baked into this design (probed on silicon):
  - DVE tensor_tensor with f32 inputs and bf16 output produces garbage ->
    multiply the bf16 copies instead
  - ACT reading bf16 input crashes the exec unit -> ACT reads f32 only
  - f32r matmul operands must be pre-rounded by a compute engine (BIR
    verifier rejects DMA-fed f32r), so bf16 stays the cheapest PE format
  - DMA cannot read PSUM -> stage through SBUF with a compute-engine copy
  - epilogue tensor ops require equal start partitions -> gather DMA first

The module is self-contained: it builds + compiles the Bass graph on first
call and caches the jitted PJRT executable for repeat calls.
"""

import numpy as np

N_FULL = 16384
D = 2048
N_CORES = 8
FSHARD = D // N_CORES  # 256 features per core
P = 128
BLOCKS = 8  # row-blocks per tile -> a tile covers 1024 batch rows
TROWS = P * BLOCKS  # 1024
N_TILES = N_FULL // TROWS  # 16
W = BLOCKS * FSHARD  # 2048 free elements per tile
NSTATS = 5
EPS = 1e-9
N_SHARD = N_FULL // N_CORES  # per-core DRAM tensor is [2048, 2048]

_CACHE = {}


def _build_nc(repeat=1, loop=None, variant="full"):
    import contextlib

    import concourse.bacc as bacc
    import concourse.tile as tile
    from concourse import mybir

    f32 = mybir.dt.float32
    bf16 = mybir.dt.bfloat16
    Act = mybir.ActivationFunctionType
    Alu = mybir.AluOpType

    nc = bacc.Bacc(
        "TRN2",
        target_bir_lowering=False,
        debug=False,
        enable_asserts=False,
        num_devices=1,
    )
    eq_d = nc.dram_tensor("e_q", [N_TILES * P, W], f32, kind="ExternalInput")
    ta_d = nc.dram_tensor("tau", [N_TILES * P, W], f32, kind="ExternalInput")
    out_d = nc.dram_tensor("out", [1, 1], f32, kind="ExternalOutput")

    with tile.TileContext(nc) as tc:
        with (
            tc.tile_pool(name="io", bufs=4) as io,
            tc.tile_pool(name="bfp", bufs=2) as bfp,
            tc.tile_pool(name="misc", bufs=1) as misc,
            tc.tile_pool(name="ep", bufs=1) as ep,
            tc.tile_pool(name="psp", bufs=1, space="PSUM") as psp,
        ):
            ones_bf = misc.tile([P, 1], bf16)
            nc.gpsimd.memset(ones_bf[:], 1.0)
            zero_b = misc.tile([P, 1], f32)
            nc.gpsimd.memset(zero_b[:], 0.0)

            # stat s lives at row 32g, cols [512 sl, 512 sl + 256) with
            # g, sl = divmod(s, 2): two banks, matmul outputs at bank starts.
            # memset once so the staging copy reads initialized memory.
            psum_stats = psp.tile([65, 1024], f32, tag="stats")
            nc.vector.memset(psum_stats[:], 0.0)

            fix_bf = None
            if variant in ("pe_only", "novec"):
                fix_bf = [
                    misc.tile([P, W], bf16, tag=f"fix{j}", name=f"fix{j}")
                    for j in range(5)
                ]
                for tbuf in fix_bf:
                    nc.vector.memset(tbuf[:], 0.01)

            for _rep in range(repeat):
                loop_cm = (
                    tc.For_i(
                        0,
                        loop,
                        1,
                        hint_engines=(
                            mybir.EngineType.PE,
                            mybir.EngineType.DVE,
                            mybir.EngineType.Activation,
                            mybir.EngineType.SP,
                        ),
                    )
                    if loop is not None
                    else contextlib.nullcontext()
                )
                with contextlib.ExitStack() as _stack:
                    _stack.enter_context(loop_cm)

                    for i in range(N_TILES):
                        if variant != "pe_only":
                            e_t = io.tile([P, W], f32, tag="e")
                            t_t = io.tile([P, W], f32, tag="t")
                            nc.sync.dma_start(e_t[:], eq_d[i * P : (i + 1) * P, :])
                            nc.sync.dma_start(t_t[:], ta_d[i * P : (i + 1) * P, :])

                        if variant == "dma_only":
                            continue

                        if variant == "full":
                            e_bf = bfp.tile([P, W], bf16, tag="e_bf")
                            t_bf = bfp.tile([P, W], bf16, tag="t_bf")
                            e2_bf = bfp.tile([P, W], bf16, tag="e2_bf")
                            t2_bf = bfp.tile([P, W], bf16, tag="t2_bf")
                            et_bf = bfp.tile([P, W], bf16, tag="et_bf")

                            nc.vector.tensor_copy(e_bf[:], e_t[:])
                            nc.vector.tensor_copy(t_bf[:], t_t[:])
                            nc.scalar.activation(
                                e2_bf[:], e_t[:], Act.Square, bias=zero_b[:]
                            )
                            nc.scalar.activation(
                                t2_bf[:], t_t[:], Act.Square, bias=zero_b[:]
                            )
                            nc.vector.tensor_mul(et_bf[:], e_bf[:], t_bf[:])
                            srcs = (e_bf, t_bf, e2_bf, t2_bf, et_bf)
                        else:
                            srcs = fix_bf

                        for s, src in enumerate(srcs):
                            g, sl = divmod(s, 2)
                            dst = psum_stats[
                                32 * g : 32 * g + 1, 512 * sl : 512 * sl + FSHARD
                            ]
                            for b in range(BLOCKS):
                                nc.tensor.matmul(
                                    dst,
                                    ones_bf[:, 0:1],
                                    src[:, b * FSHARD : (b + 1) * FSHARD],
                                    start=(i == 0 and b == 0),
                                    stop=(i == N_TILES - 1 and b == BLOCKS - 1),
                                )

                    # PSUM -> SBUF staging (DMA cannot read PSUM)
                    sb_stats = ep.tile([65, 1024], f32, tag="sb_stats")
                    nc.vector.tensor_copy(sb_stats[:], psum_stats[:])

                    # gather the five [1, 256] stats (rows {0,32,64}, slots
                    # {0, 512}) onto partition 0; slot (g2, sl1) is garbage
                    # and ignored.
                    st = ep.tile([1, 6 * FSHARD], f32, tag="st")
                    nc.sync.dma_start(
                        st[:].rearrange("p (g sl m) -> p g sl m", g=3, sl=2),
                        sb_stats[0:65:32, :].rearrange(
                            "g (sl m) -> g sl m", sl=2
                        )[:, :, :FSHARD],
                    )

                    A = st[:, 0 * FSHARD : 1 * FSHARD]  # S1e (g0, sl0)
                    B = st[:, 1 * FSHARD : 2 * FSHARD]  # S1t (g0, sl1)
                    C = st[:, 2 * FSHARD : 3 * FSHARD]  # S2e (g1, sl0)
                    Dq = st[:, 3 * FSHARD : 4 * FSHARD]  # S2t (g1, sl1)
                    E = st[:, 4 * FSHARD : 5 * FSHARD]  # Set (g2, sl0)

                    sh = [1, FSHARD]
                    zb = zero_b[0:1, 0:1]
                    aa = ep.tile(sh, f32, tag="aa")
                    bb = ep.tile(sh, f32, tag="bb")
                    ve = ep.tile(sh, f32, tag="ve")
                    vt = ep.tile(sh, f32, tag="vt")
                    stde = ep.tile(sh, f32, tag="stde")
                    stdt = ep.tile(sh, f32, tag="stdt")
                    amt = ep.tile(sh, f32, tag="amt")
                    cov = ep.tile(sh, f32, tag="cov")
                    den = ep.tile(sh, f32, tag="den")
                    rec = ep.tile(sh, f32, tag="rec")
                    cr = ep.tile(sh, f32, tag="cr")
                    ccl = ep.tile(sh, f32, tag="ccl")
                    rr = ep.tile(sh, f32, tag="rr")
                    r2 = ep.tile(sh, f32, tag="r2")
                    ls = ep.tile([1, 1], f32, tag="ls")

                    inv_n = 1.0 / N_FULL
                    # sum((x-mean)^2) = S2 - S1^2/N ; std = max(sqrt(./(N-1)), eps)
                    nc.vector.tensor_mul(aa[:], A, A)
                    nc.vector.scalar_tensor_tensor(
                        ve[:], aa[:], -inv_n, C, Alu.mult, Alu.add
                    )
                    nc.scalar.activation(
                        stde[:], ve[:], Act.Sqrt, bias=zb, scale=1.0 / (N_FULL - 1)
                    )
                    nc.vector.tensor_scalar_max(stde[:], stde[:], EPS)
                    nc.vector.tensor_mul(bb[:], B, B)
                    nc.vector.scalar_tensor_tensor(
                        vt[:], bb[:], -inv_n, Dq, Alu.mult, Alu.add
                    )
                    nc.scalar.activation(
                        stdt[:], vt[:], Act.Sqrt, bias=zb, scale=1.0 / (N_FULL - 1)
                    )
                    nc.vector.tensor_scalar_max(stdt[:], stdt[:], EPS)
                    # cov = Set - S1e*S1t/N ; c = cov / (stde*stdt) / (N+eps)
                    nc.vector.scalar_tensor_tensor(
                        amt[:], A, inv_n, B, Alu.mult, Alu.mult
                    )
                    nc.vector.tensor_sub(cov[:], E, amt[:])
                    nc.vector.tensor_mul(den[:], stde[:], stdt[:])
                    nc.vector.reciprocal(rec[:], den[:])
                    nc.vector.scalar_tensor_tensor(
                        cr[:], cov[:], 1.0 / (N_FULL + EPS), rec[:], Alu.mult, Alu.mult
                    )
                    # clip, r = 1 - c, partial loss = sum(r^2)
                    nc.vector.tensor_scalar(
                        ccl[:], cr[:], -1.0 + EPS, 1.0 - EPS, Alu.max, Alu.min
                    )
                    nc.vector.tensor_scalar(
                        rr[:], ccl[:], -1.0, 1.0, Alu.mult, Alu.add
                    )
                    nc.vector.tensor_mul(r2[:], rr[:], rr[:])
                    nc.vector.reduce_sum(ls[:], r2[:], axis=mybir.AxisListType.X)
                    nc.sync.dma_start(out_d[:], ls[:])

    nc.compile()
    return nc


class _Exec:
    """Cached PJRT executable for the SPMD kernel (mirrors
    concourse.bass2jax.run_bass_via_pjrt's multi-core branch, but keeps the
    jitted callable so repeat invocations don't recompile)."""

    def __init__(self, nc):
        import jax
        from jax.experimental.shard_map import shard_map
        from jax.sharding import Mesh, PartitionSpec

        from concourse import bass2jax, mybir

        bass2jax.install_neuronx_cc_hook()
        self.nc = nc
        partition_name = (
            nc.partition_id_tensor.name if nc.partition_id_tensor else None
        )

        in_names, out_names, out_avals, zero_outs = [], [], [], []
        for alloc in nc.m.functions[0].allocations:
            if not isinstance(alloc, mybir.MemoryLocationSet):
                continue
            assert alloc.memorylocations
            name = alloc.memorylocations[0].name
            if alloc.kind == "ExternalInput":
                if name != partition_name:
                    in_names.append(name)
            elif alloc.kind == "ExternalOutput":
                shape = tuple(alloc.tensor_shape)
                dtype = mybir.dt.np(alloc.dtype)
                out_names.append(name)
                out_avals.append(jax.core.ShapedArray(shape, dtype))
                zero_outs.append(np.zeros(shape, dtype))

        self.in_names = list(in_names)
        self.out_names = list(out_names)
        self.out_avals = out_avals
        self.zero_outs = zero_outs
        n_params = len(in_names)
        n_outs = len(out_names)

        in_names_full = list(in_names) + list(out_names)
        if partition_name is not None:
            in_names_full.append(partition_name)

        def _body(*args):
            operands = list(args)
            if partition_name is not None:
                operands.append(bass2jax.partition_id_tensor())
            outs = bass2jax._bass_exec_p.bind(
                *operands,
                out_avals=tuple(out_avals),
                in_names=tuple(in_names_full),
                out_names=tuple(out_names),
                lowering_input_output_aliases=(),
                sim_require_finite=True,
                sim_require_nnan=True,
                nc=nc,
            )
            return tuple(outs)

        devices = jax.devices()[:N_CORES]
        assert len(devices) == N_CORES, f"need {N_CORES} devices, got {len(devices)}"
        self.mesh = Mesh(np.asarray(devices), ("core",))
        in_specs = (PartitionSpec("core"),) * (n_params + n_outs)
        out_specs = (PartitionSpec("core"),) * n_outs
        donate = tuple(range(n_params, n_params + n_outs))
        self.sharded = jax.jit(
            shard_map(
                _body,
                mesh=self.mesh,
                in_specs=in_specs,
                out_specs=out_specs,
                check_rep=False,
            ),
            donate_argnums=donate,
            keep_unused=True,
        )

    def concat_zeros(self):
        return [
            np.zeros((N_CORES * z.shape[0], *z.shape[1:]), z.dtype)
            for z in self.zero_outs
        ]

    def run(self, in_map):
        """in_map: name -> full (already concat-along-axis0) array."""
        ins = [in_map[name] for name in self.in_names]
        outs = self.sharded(*ins, *self.concat_zeros())
        return {
            name: np.asarray(outs[i]).reshape(
                N_CORES, *self.out_avals[i].shape
            )
            for i, name in enumerate(self.out_names)
        }


def _get_exec(repeat=1):
    key = ("exec", repeat)
    if key not in _CACHE:
        _CACHE[key] = _Exec(_build_nc(repeat))
    return _CACHE[key]


def _shard_features(x):
    # x[16384, 2048] -> [8 * 2048, 2048] where core c's [2048, 2048] block
    # satisfies A_c[128 i + p, 256 b + m] = x[1024 i + 128 b + p, 256 c + m]
    v = x.reshape(N_TILES, BLOCKS, P, N_CORES, FSHARD)  # (i, b, p, c, m)
    v = v.transpose(3, 0, 2, 1, 4)  # (c, i, p, b, m)
    return np.ascontiguousarray(v).reshape(N_CORES * N_TILES * P, W)


def kernel(e_q, tau):
    e_q = np.ascontiguousarray(np.asarray(e_q), dtype=np.float32)
    tau = np.ascontiguousarray(np.asarray(tau), dtype=np.float32)
    assert e_q.shape == (N_FULL, D) and tau.shape == (N_FULL, D)
    ex = _get_exec()
    outs = ex.run({"e_q": _shard_features(e_q), "tau": _shard_features(tau)})
    # each core holds the partial loss over its 256 features; the sum over
    # cores is the unshard/gather step for the feature-sharded loss.
    loss = outs["out"][:, 0, 0].astype(np.float64).sum()
    return np.asarray(loss, dtype=np.float32)


# revision 11
# speedup vs baseline: 13.2990x; 1.0098x over previous
"""Trainium2 Bass kernel for BarlowTwinsLoss (nn_BarlowTwinsLoss_11038065951192).

Full inputs: e_q, tau [16384, 2048] f32. Output: scalar f32 loss.

Strategy — feature-sharded, collective-free (8 NeuronCores):
  The loss needs only 5 per-feature reductions over the batch (S1e, S1t,
  S2e, S2t, Set) plus a cheap per-feature epilogue. Instead of batch
  sharding + ReduceScatter of the D-length stats (the baseline; the 8-way
  ReduceScatter alone measured ~450 us of the 546 us pass), core c owns
  features [256c, 256c+256) and reads the FULL batch for its slice: the
  same 32 MiB/core of HBM traffic, but stats are complete locally and no
  collective is needed. Each core emits its partial loss over its 256
  features; the host sums the 8 scalars (the unshard step).

Host-side shard layout: A_c[128 i + p, 256 b + m] = x[1024 i + 128 b + p,
256 c + m], i.e. each core's [2048, 2048] f32 shard is permuted so every
[128, 2048] row-tile is DMA'd with contiguous 8 KiB partition lines (a
strided column-slice load measured 18% slower; this kernel is memory-bound,
so the DMA stream IS the runtime). All loads are triggered from the SP
HWDGE ring; the full kernel tracks the dma-only floor exactly (112 us/pass
measured for both), i.e. all compute rides under the DMA stream. A dma-only
probe with loads split across the SP + ACT HWDGE rings measured 98 us/pass
(343 GB/s/core), but in the full kernel ACT-triggered loads queue behind
the ~2 us square activations (ACT exec-queue depth is 0, so its sequencer
stalls while the engine runs) and the pass REGRESSED to 127 us; exploiting
the second ring would require first moving the squares off ACT (e.g. to
Pool as tensor_mul(x, x)) or an SWDGE ring on the idle Pool engine.

Per tile [128 rows x (8 blocks x 256 features)]:
  - DVE copies e, t to bf16; ACT squares e, t to bf16; DVE multiplies e*t
    (bf16) — all under the DMA stream
  - 40 ones-matmuls (5 stats x 8 row-blocks) of 256 output columns
    accumulate directly into one [1, 256] PSUM slot per stat (matmul
    outputs may only target PSUM partitions {0,32,64}; stat s -> row 32g,
    bank-start col 512 sl with g, sl = divmod(s, 2)); no block fold needed
Tail (once): one [65, 1024] PSUM->SBUF staging copy, one SBUF->SBUF gather
DMA that brings the 5 stats to partition 0, then the 256-feature epilogue
(mean/var/corr/clip/loss) and a [1,1] DMA out.

Hardware pitfalls baked into this design (probed on silicon):
  - DVE tensor_tensor with f32 inputs and bf16 output produces garbage ->
    multiply the bf16 copies instead
  - ACT reading bf16 input crashes the exec unit -> ACT reads f32 only
  - f32r matmul operands must be pre-rounded by a compute engine (BIR
    verifier rejects DMA-fed f32r), so bf16 stays the cheapest PE format
  - DMA cannot read PSUM -> stage through SBUF with a compute-engine copy
  - epilogue tensor ops require equal start partitions -> gather DMA first

The module is self-contained: it builds + compiles the Bass graph on first
call and caches the jitted PJRT executable for repeat calls.
"""

import numpy as np

N_FULL = 16384
D = 2048
N_CORES = 8
FSHARD = D // N_CORES  # 256 features per core
P = 128
BLOCKS = 8  # row-blocks per tile -> a tile covers 1024 batch rows
TROWS = P * BLOCKS  # 1024
N_TILES = N_FULL // TROWS  # 16
W = BLOCKS * FSHARD  # 2048 free elements per tile
NSTATS = 5
EPS = 1e-9
N_SHARD = N_FULL // N_CORES  # per-core DRAM tensor is [2048, 2048]

_CACHE = {}


def _build_nc(repeat=1, loop=None, variant="full"):
    import contextlib

    import concourse.bacc as bacc
    import concourse.tile as tile
    from concourse import mybir

    f32 = mybir.dt.float32
    bf16 = mybir.dt.bfloat16
    Act = mybir.ActivationFunctionType
    Alu = mybir.AluOpType

    nc = bacc.Bacc(
        "TRN2",
        target_bir_lowering=False,
        debug=False,
        enable_asserts=False,
        num_devices=1,
    )
    eq_d = nc.dram_tensor("e_q", [N_TILES * P, W], f32, kind="ExternalInput")
    ta_d = nc.dram_tensor("tau", [N_TILES * P, W], f32, kind="ExternalInput")
    out_d = nc.dram_tensor("out", [1, 1], f32, kind="ExternalOutput")

    with tile.TileContext(nc) as tc:
        with (
            tc.tile_pool(name="io", bufs=4) as io,
            tc.tile_pool(name="bfp", bufs=2) as bfp,
            tc.tile_pool(name="misc", bufs=1) as misc,
            tc.tile_pool(name="ep", bufs=1) as ep,
            tc.tile_pool(name="psp", bufs=1, space="PSUM") as psp,
        ):
            ones_bf = misc.tile([P, 1], bf16)
            nc.gpsimd.memset(ones_bf[:], 1.0)
            zero_b = misc.tile([P, 1], f32)
            nc.gpsimd.memset(zero_b[:], 0.0)

            # stat s lives at row 32g, cols [512 sl, 512 sl + 256) with
            # g, sl = divmod(s, 2): two banks, matmul outputs at bank starts.
            # memset once so the staging copy reads initialized memory.
            psum_stats = psp.tile([65, 1024], f32, tag="stats")
            nc.vector.memset(psum_stats[:], 0.0)

            fix_bf = None
            if variant in ("pe_only", "novec"):
                fix_bf = [
                    misc.tile([P, W], bf16, tag=f"fix{j}", name=f"fix{j}")
                    for j in range(5)
                ]
                for tbuf in fix_bf:
                    nc.vector.memset(tbuf[:], 0.01)

            for _rep in range(repeat):
                loop_cm = (
                    tc.For_i(
                        0,
                        loop,
                        1,
                        hint_engines=(
                            mybir.EngineType.PE,
                            mybir.EngineType.DVE,
                            mybir.EngineType.Activation,
                            mybir.EngineType.SP,
                        ),
                    )
                    if loop is not None
                    else contextlib.nullcontext()
                )
                with contextlib.ExitStack() as _stack:
                    _stack.enter_context(loop_cm)

                    for i in range(N_TILES):
                        if variant != "pe_only":
                            e_t = io.tile([P, W], f32, tag="e")
                            t_t = io.tile([P, W], f32, tag="t")
                            nc.sync.dma_start(e_t[:], eq_d[i * P : (i + 1) * P, :])
                            nc.sync.dma_start(t_t[:], ta_d[i * P : (i + 1) * P, :])

                        if variant == "dma_only":
                            continue

                        if variant == "full":
                            e_bf = bfp.tile([P, W], bf16, tag="e_bf")
                            t_bf = bfp.tile([P, W], bf16, tag="t_bf")
                            e2_bf = bfp.tile([P, W], bf16, tag="e2_bf")
                            t2_bf = bfp.tile([P, W], bf16, tag="t2_bf")
                            et_bf = bfp.tile([P, W], bf16, tag="et_bf")

                            nc.vector.tensor_copy(e_bf[:], e_t[:])
                            nc.vector.tensor_copy(t_bf[:], t_t[:])
                            nc.scalar.activation(
                                e2_bf[:], e_t[:], Act.Square, bias=zero_b[:]
                            )
                            nc.scalar.activation(
                                t2_bf[:], t_t[:], Act.Square, bias=zero_b[:]
                            )
                            nc.vector.tensor_mul(et_bf[:], e_bf[:], t_bf[:])
                            srcs = (e_bf, t_bf, e2_bf, t2_bf, et_bf)
                        else:
                            srcs = fix_bf

                        for s, src in enumerate(srcs):
                            g, sl = divmod(s, 2)
                            dst = psum_stats[
                                32 * g : 32 * g + 1, 512 * sl : 512 * sl + FSHARD
                            ]
                            for b in range(BLOCKS):
                                nc.tensor.matmul(
                                    dst,
                                    ones_bf[:, 0:1],
                                    src[:, b * FSHARD : (b + 1) * FSHARD],
                                    start=(i == 0 and b == 0),
                                    stop=(i == N_TILES - 1 and b == BLOCKS - 1),
                                )

                    # PSUM -> SBUF staging (DMA cannot read PSUM)
                    sb_stats = ep.tile([65, 1024], f32, tag="sb_stats")
                    nc.vector.tensor_copy(sb_stats[:], psum_stats[:])

                    # gather the five [1, 256] stats (rows {0,32,64}, slots
                    # {0, 512}) onto partition 0; slot (g2, sl1) is garbage
                    # and ignored.
                    st = ep.tile([1, 6 * FSHARD], f32, tag="st")
                    nc.sync.dma_start(
                        st[:].rearrange("p (g sl m) -> p g sl m", g=3, sl=2),
                        sb_stats[0:65:32, :].rearrange(
                            "g (sl m) -> g sl m", sl=2
                        )[:, :, :FSHARD],
                    )

                    A = st[:, 0 * FSHARD : 1 * FSHARD]  # S1e (g0, sl0)
                    B = st[:, 1 * FSHARD : 2 * FSHARD]  # S1t (g0, sl1)
                    C = st[:, 2 * FSHARD : 3 * FSHARD]  # S2e (g1, sl0)
                    Dq = st[:, 3 * FSHARD : 4 * FSHARD]  # S2t (g1, sl1)
                    E = st[:, 4 * FSHARD : 5 * FSHARD]  # Set (g2, sl0)

                    sh = [1, FSHARD]
                    zb = zero_b[0:1, 0:1]
                    aa = ep.tile(sh, f32, tag="aa")
                    bb = ep.tile(sh, f32, tag="bb")
                    ve = ep.tile(sh, f32, tag="ve")
                    vt = ep.tile(sh, f32, tag="vt")
                    stde = ep.tile(sh, f32, tag="stde")
                    stdt = ep.tile(sh, f32, tag="stdt")
                    amt = ep.tile(sh, f32, tag="amt")
                    cov = ep.tile(sh, f32, tag="cov")
                    den = ep.tile(sh, f32, tag="den")
                    rec = ep.tile(sh, f32, tag="rec")
                    cr = ep.tile(sh, f32, tag="cr")
                    ccl = ep.tile(sh, f32, tag="ccl")
                    rr = ep.tile(sh, f32, tag="rr")
                    r2 = ep.tile(sh, f32, tag="r2")
                    ls = ep.tile([1, 1], f32, tag="ls")

                    inv_n = 1.0 / N_FULL
                    # sum((x-mean)^2) = S2 - S1^2/N ; std = max(sqrt(./(N-1)), eps)
                    nc.vector.tensor_mul(aa[:], A, A)
                    nc.vector.scalar_tensor_tensor(
                        ve[:], aa[:], -inv_n, C, Alu.mult, Alu.add
                    )
                    nc.scalar.activation(
                        stde[:], ve[:], Act.Sqrt, bias=zb, scale=1.0 / (N_FULL - 1)
                    )
                    nc.vector.tensor_scalar_max(stde[:], stde[:], EPS)
                    nc.vector.tensor_mul(bb[:], B, B)
                    nc.vector.scalar_tensor_tensor(
                        vt[:], bb[:], -inv_n, Dq, Alu.mult, Alu.add
                    )
                    nc.scalar.activation(
                        stdt[:], vt[:], Act.Sqrt, bias=zb, scale=1.0 / (N_FULL - 1)
                    )
                    nc.vector.tensor_scalar_max(stdt[:], stdt[:], EPS)
                    # cov = Set - S1e*S1t/N ; c = cov / (stde*stdt) / (N+eps)
                    nc.vector.scalar_tensor_tensor(
                        amt[:], A, inv_n, B, Alu.mult, Alu.mult
                    )
                    nc.vector.tensor_sub(cov[:], E, amt[:])
                    nc.vector.tensor_mul(den[:], stde[:], stdt[:])
                    nc.vector.reciprocal(rec[:], den[:])
                    nc.vector.scalar_tensor_tensor(
                        cr[:], cov[:], 1.0 / (N_FULL + EPS), rec[:], Alu.mult, Alu.mult
                    )
                    # clip, r = 1 - c, partial loss = sum(r^2)
                    nc.vector.tensor_scalar(
                        ccl[:], cr[:], -1.0 + EPS, 1.0 - EPS, Alu.max, Alu.min
                    )
                    nc.vector.tensor_scalar(
                        rr[:], ccl[:], -1.0, 1.0, Alu.mult, Alu.add
                    )
                    nc.vector.tensor_mul(r2[:], rr[:], rr[:])
                    nc.vector.reduce_sum(ls[:], r2[:], axis=mybir.AxisListType.X)
                    nc.sync.dma_start(out_d[:], ls[:])

    nc.compile()
    return nc


class _Exec:
    """Cached PJRT executable for the SPMD kernel (mirrors
    concourse.bass2jax.run_bass_via_pjrt's multi-core branch, but keeps the
    jitted callable so repeat invocations don't recompile)."""

    def __init__(self, nc):
        import jax
        from jax.experimental.shard_map import shard_map
        from jax.sharding import Mesh, PartitionSpec

        from concourse import bass2jax, mybir

        bass2jax.install_neuronx_cc_hook()
        self.nc = nc
        partition_name = (
            nc.partition_id_tensor.name if nc.partition_id_tensor else None
        )

        in_names, out_names, out_avals, zero_outs = [], [], [], []
        for alloc in nc.m.functions[0].allocations:
            if not isinstance(alloc, mybir.MemoryLocationSet):
                continue
            assert alloc.memorylocations
            name = alloc.memorylocations[0].name
            if alloc.kind == "ExternalInput":
                if name != partition_name:
                    in_names.append(name)
            elif alloc.kind == "ExternalOutput":
                shape = tuple(alloc.tensor_shape)
                dtype = mybir.dt.np(alloc.dtype)
                out_names.append(name)
                out_avals.append(jax.core.ShapedArray(shape, dtype))
                zero_outs.append(np.zeros(shape, dtype))

        self.in_names = list(in_names)
        self.out_names = list(out_names)
        self.out_avals = out_avals
        self.zero_outs = zero_outs
        n_params = len(in_names)
        n_outs = len(out_names)

        in_names_full = list(in_names) + list(out_names)
        if partition_name is not None:
            in_names_full.append(partition_name)

        def _body(*args):
            operands = list(args)
            if partition_name is not None:
                operands.append(bass2jax.partition_id_tensor())
            outs = bass2jax._bass_exec_p.bind(
                *operands,
                out_avals=tuple(out_avals),
                in_names=tuple(in_names_full),
                out_names=tuple(out_names),
                lowering_input_output_aliases=(),
                sim_require_finite=True,
                sim_require_nnan=True,
                nc=nc,
            )
            return tuple(outs)

        devices = jax.devices()[:N_CORES]
        assert len(devices) == N_CORES, f"need {N_CORES} devices, got {len(devices)}"
        self.mesh = Mesh(np.asarray(devices), ("core",))
        in_specs = (PartitionSpec("core"),) * (n_params + n_outs)
        out_specs = (PartitionSpec("core"),) * n_outs
        donate = tuple(range(n_params, n_params + n_outs))
        self.sharded = jax.jit(
            shard_map(
                _body,
                mesh=self.mesh,
                in_specs=in_specs,
                out_specs=out_specs,
                check_rep=False,
            ),
            donate_argnums=donate,
            keep_unused=True,
        )

    def concat_zeros(self):
        return [
            np.zeros((N_CORES * z.shape[0], *z.shape[1:]), z.dtype)
            for z in self.zero_outs
        ]

    def run(self, in_map):
        """in_map: name -> full (already concat-along-axis0) array."""
        ins = [in_map[name] for name in self.in_names]
        outs = self.sharded(*ins, *self.concat_zeros())
        return {
            name: np.asarray(outs[i]).reshape(
                N_CORES, *self.out_avals[i].shape
            )
            for i, name in enumerate(self.out_names)
        }


def _get_exec(repeat=1):
    key = ("exec", repeat)
    if key not in _CACHE:
        _CACHE[key] = _Exec(_build_nc(repeat))
    return _CACHE[key]


def _shard_features(x):
    # x[16384, 2048] -> [8 * 2048, 2048] where core c's [2048, 2048] block
    # satisfies A_c[128 i + p, 256 b + m] = x[1024 i + 128 b + p, 256 c + m]
    v = x.reshape(N_TILES, BLOCKS, P, N_CORES, FSHARD)  # (i, b, p, c, m)
    v = v.transpose(3, 0, 2, 1, 4)  # (c, i, p, b, m)
    return np.ascontiguousarray(v).reshape(N_CORES * N_TILES * P, W)


def kernel(e_q, tau):
    e_q = np.ascontiguousarray(np.asarray(e_q), dtype=np.float32)
    tau = np.ascontiguousarray(np.asarray(tau), dtype=np.float32)
    assert e_q.shape == (N_FULL, D) and tau.shape == (N_FULL, D)
    ex = _get_exec()
    outs = ex.run({"e_q": _shard_features(e_q), "tau": _shard_features(tau)})
    # each core holds the partial loss over its 256 features; the sum over
    # cores is the unshard/gather step for the feature-sharded loss.
    loss = outs["out"][:, 0, 0].astype(np.float64).sum()
    return np.asarray(loss, dtype=np.float32)
